# revision 27
# baseline (speedup 1.0000x reference)
# Multi-head attention (B=2, S=2048, D=1024, H=16) on 8 trn2 NeuronCores.
#
# Sharding: core c -> batch b=c//4, head-group g=c%4 (heads 4g..4g+3).
#   - Q/K/V projections column-split (256 cols per core), O projection
#     row-split (256 rows per core) per Megatron; partial outputs summed
#     on host (4 cores per batch).
# Per-core layout strategy ("transposed attention"):
#   - xT = x.T via PE transpose (fp32r, full-rate fp32)
#   - qT,kT [d, S] via lhsT=W chunks, rhs=xT   (fp32r)
#   - scores_T [k, q] = kT.T-chunk @ qT        (fp32r, exact fp32)
#   - p_T = exp(scores_T * 0.125) on ScalarE -> bf16 SBUF
#   - ctx_T [65, q] = v_aug.T @ p_T  accumulated over k-chunks; the 65th
#     row of v_aug is ones -> row 64 of ctx_T is the softmax denominator
#   - attn output: PE-transpose p_T blocks back to [q, k] (bf16), then
#     normalize with per-partition 1/denom while copying PSUM->SBUF f32
#   - out_part [S, 1024] = ctxn.T-chunks @ Wo-rows (fp32r)
# Assumptions baked in from the problem spec: mask is all-ones (ignored),
# biases are all-zero (bq/bk/bv ignored; bo added on host).

import numpy as np

S = 2048
D = 1024
DC = 256  # cols (and rows for Wo) per core = 4 heads * 64
DH = 64
N_CORES = 8

_CACHE = {}


def _build():
    from contextlib import ExitStack

    import concourse.bacc as bacc
    import concourse.mybir as mybir
    import concourse.tile as tile
    from concourse.masks import make_identity

    f32 = mybir.dt.float32
    f32r = mybir.dt.float32r
    bf16 = mybir.dt.bfloat16

    nc = bacc.Bacc("TRN2", target_bir_lowering=False, debug=False)

    xq = nc.dram_tensor("xq", [S, D], f32, kind="ExternalInput").ap()
    xk = nc.dram_tensor("xk", [S, D], f32, kind="ExternalInput").ap()
    xv = nc.dram_tensor("xv", [S, D], f32, kind="ExternalInput").ap()
    wq = nc.dram_tensor("wq", [D, DC], f32, kind="ExternalInput").ap()
    wk = nc.dram_tensor("wk", [D, DC], f32, kind="ExternalInput").ap()
    wv = nc.dram_tensor("wv", [D, DC], f32, kind="ExternalInput").ap()
    wo = nc.dram_tensor("wo", [DC, D], f32, kind="ExternalInput").ap()
    attn_out = nc.dram_tensor("attn_part", [4, S, S], bf16, kind="ExternalOutput").ap()
    out_part = nc.dram_tensor("out_part", [S, D], f32, kind="ExternalOutput").ap()

    with tile.TileContext(nc) as tc, ExitStack() as ctx:
        consts = ctx.enter_context(tc.tile_pool(name="consts", bufs=1))
        resident = ctx.enter_context(tc.tile_pool(name="resident", bufs=1))
        p1sb = ctx.enter_context(tc.tile_pool(name="p1sb", bufs=2))
        xnatp = ctx.enter_context(tc.tile_pool(name="xnatp", bufs=2))
        p2sb = ctx.enter_context(tc.tile_pool(name="p2sb", bufs=2))
        pTpool = ctx.enter_context(tc.tile_pool(name="pTpool", bufs=2))
        stagep = ctx.enter_context(tc.tile_pool(name="stage", bufs=3))
        ps_sp = ctx.enter_context(tc.tile_pool(name="ps_s", bufs=2, space="PSUM"))
        ps_pp = ctx.enter_context(tc.tile_pool(name="ps_p", bufs=2, space="PSUM"))
        ps_cp = ctx.enter_context(tc.tile_pool(name="ps_c", bufs=2, space="PSUM"))

        # --- constants ---
        wq_sb = consts.tile([128, 8, DC], f32r, tag="wq")
        wk_sb = consts.tile([128, 8, DC], f32r, tag="wk")
        wv_sb = consts.tile([128, 8, DC], f32r, tag="wv")
        wo_sb = consts.tile([128, 2, D], bf16, tag="wo")
        ident_f = consts.tile([128, 128], f32, tag="idf")
        ident_b = consts.tile([128, 128], bf16, tag="idb")
        make_identity(nc, ident_f)
        make_identity(nc, ident_b)
        ones1 = consts.tile([1, 64], f32, tag="ones")
        nc.vector.memset(ones1, 1.0)
        ones_b = consts.tile([1, 64], bf16, tag="onesb")
        nc.vector.memset(ones_b, 1.0)

        # --- resident activations ---
        qT = resident.tile([128, 2, 4, 512], f32r, tag="qT")  # [dcol%128, dc, sc, tok]
        kT = resident.tile([128, 2, 4, 512], f32r, tag="kT")
        # v_aug: head h at cols [65h, 65h+64], ones at col 65h+64
        v_aug = resident.tile([128, 16, 260], bf16, tag="vaug")
        ctxn = resident.tile([128, 2, 4, 512], bf16, tag="ctxn")
        for h in range(4):
            nc.vector.memset(v_aug[:, :, 65 * h + 64], 1.0)

        # ---------- Phase 1: transposes + projections ----------
        # input order key, value, query: phase-2 unit (qb) unlocks as soon
        # as the matching query chunk is projected.
        for inp_i, (x_dram, w_sb) in enumerate(
            [(xk, wk_sb), (xv, wv_sb), (xq, wq_sb)]
        ):
            for sc in range(4):
                x_nat = xnatp.tile([128, 4, D], f32, tag="xnat")
                for a in range(4):
                    nc.sync.dma_start(
                        out=x_nat[:, a, :],
                        in_=x_dram[
                            sc * 512 + a * 128 : sc * 512 + (a + 1) * 128, :
                        ],
                    )
                it = inp_i * 4 + sc
                if it < 4:
                    # weight loads staggered between x chunks; staging slot
                    # borrowed from the xT tag, rounded to f32r/bf16 by a copy
                    w_dram2, w_sb2 = [
                        (wk, wk_sb), (wv, wv_sb), (wq, wq_sb), (wo, wo_sb)
                    ][it]
                    st = p1sb.tile(list(w_sb2.shape[:-1]) + [w_sb2.shape[-1]], f32, tag="xT")
                    nc.sync.dma_start(
                        out=st, in_=w_dram2.rearrange("(a p) c -> p a c", p=128)
                    )
                    nc.vector.tensor_copy(w_sb2, st)
                xT = p1sb.tile([128, 8, 512], f32r, tag="xT")
                for mb in range(8):
                    ps_t = ps_pp.tile([128, 512], f32, tag="psp")
                    for a in range(4):
                        nc.tensor.transpose(
                            ps_t[:, a * 128 : (a + 1) * 128],
                            x_nat[:, a, mb * 128 : (mb + 1) * 128],
                            ident_f,
                        )
                    if mb % 3 == 0:
                        nc.vector.tensor_copy(xT[:, mb, :], ps_t)
                    else:
                        nc.scalar.copy(xT[:, mb, :], ps_t)

                if inp_i != 1:  # key/query -> transposed projection
                    dst = kT if inp_i == 0 else qT
                    for dc in range(2):
                        ps_q = ps_sp.tile([128, 512], f32, tag="pss")
                        for mb in range(8):
                            nc.tensor.matmul(
                                ps_q,
                                w_sb[:, mb, dc * 128 : (dc + 1) * 128],
                                xT[:, mb, :],
                                start=(mb == 0),
                                stop=(mb == 7),
                            )
                        nc.vector.tensor_copy(dst[:, dc, sc, :], ps_q)
                else:  # value -> natural layout, cast bf16, per-head cols
                    for tc4 in range(4):
                        ps_v = ps_sp.tile([128, DC], f32, tag="pss")
                        for mb in range(8):
                            nc.tensor.matmul(
                                ps_v,
                                xT[:, mb, tc4 * 128 : (tc4 + 1) * 128],
                                w_sb[:, mb, :],
                                start=(mb == 0),
                                stop=(mb == 7),
                            )
                        kc = sc * 4 + tc4
                        nc.vector.tensor_copy(
                            v_aug[:, kc, :].rearrange("p (h c) -> p h c", h=4)[
                                :, :, 0:64
                            ],
                            ps_v.rearrange("p (h c) -> p h c", h=4),
                        )

        # ---------- Phase 2+3: attention, out-proj folded per q-block ----------
        def emit_attn_out(qs_list, h, qb, p_T, recipT):
            # transpose p_T back to [q, k]; normalized bf16 attn
            for qs in qs_list:
                stage = stagep.tile([128, S], bf16, tag="stage")
                for g in range(2):
                    ps_p = ps_pp.tile([128, 1024], bf16, tag="psp")
                    for j in range(8):
                        kc = g * 8 + j
                        nc.tensor.transpose(
                            ps_p[:, j * 128 : (j + 1) * 128],
                            p_T[:, kc, qs * 128 : (qs + 1) * 128],
                            ident_b,
                        )
                    nc.vector.tensor_scalar_mul(
                        stage[:, g * 1024 : (g + 1) * 1024],
                        ps_p,
                        recipT[:, qs : qs + 1],
                    )
                nc.sync.dma_start(
                    out=attn_out[
                        h,
                        qb * 512 + qs * 128 : qb * 512 + qs * 128 + 128,
                        :,
                    ],
                    in_=stage,
                )

        def emit_oproj(qb):
            # out-projection for this q-block (all heads ready)
            for sub in range(4):
                o_stage = stagep.tile([128, D], f32, tag="stage")
                for nb in range(2):
                    ps_o = ps_sp.tile([128, 512], f32, tag="pss")
                    for cc in range(2):
                        nc.tensor.matmul(
                            ps_o,
                            ctxn[:, cc, qb, sub * 128 : (sub + 1) * 128],
                            wo_sb[:, cc, nb * 512 : (nb + 1) * 512],
                            start=(cc == 0),
                            stop=(cc == 1),
                        )
                    nc.vector.tensor_copy(o_stage[:, nb * 512 : (nb + 1) * 512], ps_o)
                row = qb * 512 + sub * 128
                nc.sync.dma_start(out=out_part[row : row + 128, :], in_=o_stage)

        def emit_head_epilogue(h, qb, p_T):
            dc, r0 = h // 2, 64 * (h % 2)
            ps_c = ps_cp.tile([128, 512], f32, tag="psc")
            for kc in range(16):
                nc.tensor.matmul(
                    ps_c[:65, :],
                    v_aug[:, kc, 65 * h : 65 * h + 65],
                    p_T[:, kc, :],
                    start=(kc == 0),
                    stop=(kc == 15),
                )
            recip = p2sb.tile([1, 512], f32, tag="recip")
            nc.vector.reciprocal(recip, ps_c[64:65, :])
            # per-partition copies of recip: [128q, 1] x 4
            ps_r = ps_pp.tile([128, 4], f32, tag="psp")
            for j in range(4):
                nc.tensor.matmul(
                    ps_r[:, j : j + 1],
                    recip[0:1, j * 128 : (j + 1) * 128],
                    ones1[0:1, 0:1],
                    start=True,
                    stop=True,
                )
            recipT = p2sb.tile([128, 4], f32, tag="recipT")
            nc.vector.tensor_copy(recipT, ps_r)
            # broadcast recip across 64 partitions for ctx normalize (fp32:
            # the 4 cyc/row penalty on [1,64]x[1,512] is ~0.6us/head, and
            # keeping the multiplier exact saves ~1.5e-3 of out error)
            ps_b = ps_pp.tile([64, 512], f32, tag="psp")
            nc.tensor.matmul(ps_b, ones1, recip, start=True, stop=True)
            bcast = p2sb.tile([64, 512], f32, tag="bcast")
            nc.vector.tensor_copy(bcast, ps_b)
            nc.vector.tensor_mul(ctxn[r0 : r0 + 64, dc, qb, :], ps_c[:64, :], bcast)

            if h == 3:
                emit_attn_out([0, 1], h, qb, p_T, recipT)
                emit_oproj(qb)
                emit_attn_out([2, 3], h, qb, p_T, recipT)
            else:
                emit_attn_out([0, 1, 2, 3], h, qb, p_T, recipT)

        for qb in range(4):
            for h in range(4):
                dc, r0 = h // 2, 64 * (h % 2)
                p_T = pTpool.tile([128, 16, 512], bf16, tag="pT")
                for kc2 in range(8):
                    ps_s = ps_sp.tile([128, 1024], f32, tag="pss")
                    for j in range(2):
                        kc = 2 * kc2 + j
                        nc.tensor.matmul(
                            ps_s[:, j * 512 : (j + 1) * 512],
                            kT[
                                r0 : r0 + 64, dc, kc // 4,
                                (kc % 4) * 128 : (kc % 4) * 128 + 128,
                            ],
                            qT[r0 : r0 + 64, dc, qb, :],
                            start=True,
                            stop=True,
                        )
                    nc.scalar.activation(
                        p_T[:, 2 * kc2 : 2 * kc2 + 2, :].rearrange("p a b -> p (a b)"),
                        ps_s,
                        mybir.ActivationFunctionType.Exp,
                        scale=0.125,
                    )
                emit_head_epilogue(h, qb, p_T)

    nc.compile()
    return nc


def _get_nc():
    if "nc" not in _CACHE:
        _CACHE["nc"] = _build()
    return _CACHE["nc"]


def _make_in_maps(query, key, value, Wq, Wk, Wv, Wo):
    asf = lambda a: np.ascontiguousarray(np.asarray(a, dtype=np.float32))
    in_maps = []
    for c in range(N_CORES):
        b, g = divmod(c, 4)
        in_maps.append(
            {
                "xq": asf(query[b]),
                "xk": asf(key[b]),
                "xv": asf(value[b]),
                "wq": asf(Wq[:, DC * g : DC * (g + 1)]),
                "wk": asf(Wk[:, DC * g : DC * (g + 1)]),
                "wv": asf(Wv[:, DC * g : DC * (g + 1)]),
                "wo": asf(Wo[DC * g : DC * (g + 1), :]),
            }
        )
    return in_maps


def _gather(results, bo):
    B = 2
    H = 16
    attn = np.empty((B, H, S, S), dtype=np.float32)
    out = np.zeros((B, S, D), dtype=np.float32)
    for c in range(N_CORES):
        b, g = divmod(c, 4)
        attn[b, 4 * g : 4 * g + 4] = np.asarray(
            results[c]["attn_part"], dtype=np.float32
        ).reshape(4, S, S)
        out[b] += results[c]["out_part"]
    out += np.asarray(bo, dtype=np.float32)
    return out, attn


def run(inputs, trace=False):
    from concourse.bass_utils import run_bass_kernel_spmd

    nc = _get_nc()
    in_maps = _make_in_maps(
        inputs["query"], inputs["key"], inputs["value"],
        inputs["Wq"], inputs["Wk"], inputs["Wv"], inputs["Wo"],
    )
    res = run_bass_kernel_spmd(
        nc, in_maps, core_ids=list(range(N_CORES)), trace=trace
    )
    out, attn = _gather(res.results, inputs["bo"])
    return (out, attn), res


def kernel(**inputs):
    (out, attn), _ = run(inputs, trace=False)
    return out, attn
